# revision 8
# baseline (speedup 1.0000x reference)
"""Trainium2 Bass kernel for nn_CompactControlAttention.

The module's attention is degenerate: softmax over a size-1 axis is exactly
1.0, so queries/keys (Wq, bq, Wk, bk) never affect the output:

    out[b, s, :] = sequence[b, s, :] + p[b, :]
    p = (sum_c controls[c]) @ Wv.T @ Wo.T + C * (bv @ Wo.T + bo)

Sharding (8 cores, no collectives): tensor-parallel over the OUTPUT feature
dim e.  Core k computes out[:, :, 256k:256(k+1)], needing full Wv
(replicated), a 256-column slice of Wo, and matching slices of seq/bo.
Cross-core collectives were measured to pay a 50-70us first-collective
rendezvous on this runtime (core launch skew), so the kernel is
communication-free and optimized as a single streaming pipeline:

  - All tensors are pre-packed host-side into SBUF layout ([128, free],
    partition-contiguous) so every DMA moves at line rate (~410 GB/s).
  - ctrl (bf16) streams in 4 sub-chunks; the fold over C (bf16 pair-adds,
    alternating DVE/GPSIMD) and the stacked-identity transpose-matmul
    (folds the last c-parity pair) pipeline against the stream.
  - A dummy-matmul burst right after the first ctrl chunk pulls the PE out
    of the HAM 1.2 GHz throttle before the real GEMM work arrives.
  - Wv.T streams as 4 column panels x 4 sub-chunks; MM1 K-tiles fire per
    sub-chunk.  Each panel's bias/cast/transposes/MM2 overlap the next
    panel's stream.  PSUM is quad-buffered so no panel ever waits on a
    downstream consumer.
  - Engine split: PE does all matmuls/transposes; DVE and GPSIMD split the
    folds and the final residual adds; ACT does every PSUM->SBUF cast (v,
    csT blocks, p) plus the seq fp8->f32 casts, interleaved into the panel
    loop; output writes stream on the scalar (ACT HWDGE) queue.

Precision: bf16 weights/activations with fp32 PSUM accumulation; sequence
ships as fp8e4m3 (it only enters the final residual add; adds ~1e-3 of
output-scale rounding).  Measured rel err ~4e-3 vs the fp32 reference.
"""

import numpy as np
import ml_dtypes

import concourse.bass as bass
import concourse.mybir as mybir
import concourse.tile as tile
from concourse import bacc
from concourse.bass_utils import run_bass_kernel_spmd
from concourse.masks import make_identity

N_CORES = 8
D = 2048
B = 64
S = 32
C = 8
EK = D // N_CORES  # 256: per-core output-feature slice
F32 = mybir.dt.float32
BF16 = mybir.dt.bfloat16
FP8 = mybir.dt.float8e4
P = 128
NPANEL = 4
PW = D // NPANEL  # 512 v-features per panel
NSUB = 4  # sub-DMAs per panel (4 K-tiles each)
NCC = 4  # ctrl sub-chunks (512 d-cols each)
CD = D // NCC  # 512
COPY = mybir.ActivationFunctionType.Copy

_CACHE = {}


def _build_nc():
    nc = bacc.Bacc("TRN2", target_bir_lowering=False, debug=False, num_devices=N_CORES)

    # All inputs pre-packed host-side to SBUF layout [128, free].
    seq = nc.dram_tensor("seq", [P, S * P], FP8, kind="ExternalInput")
    ctrl = nc.dram_tensor("ctrl", [P, 4 * D], BF16, kind="ExternalInput")
    wvp = nc.dram_tensor("wvp", [NPANEL * P, 16 * PW], BF16, kind="ExternalInput")
    wot = nc.dram_tensor("wot", [P, 16 * EK], BF16, kind="ExternalInput")
    bv = nc.dram_tensor("bv", [1, D], F32, kind="ExternalInput")
    bo = nc.dram_tensor("bo", [1, EK], F32, kind="ExternalInput")
    out = nc.dram_tensor("out", [P, S * P], F32, kind="ExternalOutput")

    with tile.TileContext(nc) as tc:
        _body(tc, seq, ctrl, wvp, wot, bv, bo, out)
    nc.compile()
    return nc


def _body(tc, seq, ctrl, wvp, wot, bv, bo, out):
    from contextlib import ExitStack

    ctx = ExitStack()
    nc = tc.nc

    consts = ctx.enter_context(tc.tile_pool(name="consts", bufs=1))
    sbuf = ctx.enter_context(tc.tile_pool(name="sbuf", bufs=1))
    wpool = ctx.enter_context(tc.tile_pool(name="wv", bufs=2))
    psum_t = ctx.enter_context(tc.tile_pool(name="psum_t", bufs=2, space="PSUM"))
    psum_v = ctx.enter_context(tc.tile_pool(name="psum_v", bufs=4, space="PSUM"))
    psum_p = ctx.enter_context(tc.tile_pool(name="psum_p", bufs=1, space="PSUM"))
    psum_d = ctx.enter_context(tc.tile_pool(name="psum_d", bufs=1, space="PSUM"))

    # --- constants: tiny DMAs at the FRONT of the sync queue --------------
    ident = consts.tile([P, P], F32)
    make_identity(nc, ident[:])
    sel_f = consts.tile([P, B], F32)  # two stacked 64x64 identities
    nc.sync.dma_start(out=sel_f[0:B, :], in_=ident[0:B, 0:B])
    nc.sync.dma_start(out=sel_f[B : 2 * B, :], in_=ident[0:B, 0:B])
    bv_f = consts.tile([1, D], F32)
    nc.sync.dma_start(out=bv_f[:], in_=bv[:])
    bo_f = consts.tile([1, EK], F32)
    nc.sync.dma_start(out=bo_f[:], in_=bo[:])

    # --- sync (SP) queue: ctrl x4, p0s0, p0s1, wot, p0s2.., seq on gpsimd -
    ctrl_sb = sbuf.tile([P, 4 * D], BF16)
    for c in range(NCC):
        nc.sync.dma_start(
            out=ctrl_sb[:, c * 4 * CD : (c + 1) * 4 * CD],
            in_=ctrl[:, c * 4 * CD : (c + 1) * 4 * CD],
        )

    wv_sb = []
    wo_sb = sbuf.tile([P, 16 * EK], BF16)
    for c in range(NPANEL):
        w = wpool.tile([P, 16 * PW], BF16, name=f"wvpanel{c % 2}", tag=f"wv{c % 2}")
        wv_sb.append(w)
        for s in range(NSUB):
            sl = slice(s * 4 * PW, (s + 1) * 4 * PW)
            nc.sync.dma_start(out=w[:, sl], in_=wvp[c * P : (c + 1) * P, sl])
            if c == 0 and s == 1:  # wot mid-panel-0: arrives before MM2-p0
                nc.sync.dma_start(out=wo_sb[:], in_=wot[:])
    seq_sb = sbuf.tile([P, S * P], FP8)
    nc.gpsimd.dma_start(out=seq_sb[:], in_=seq[:])

    # --- small on-engine constants ---------------------------------------
    sel = consts.tile([P, B], BF16)
    nc.vector.tensor_copy(sel[:], sel_f[:])
    ident_t = consts.tile([B, B], BF16)
    nc.vector.tensor_copy(ident_t[:], ident[0:B, 0:B])
    onesC_f = consts.tile([1, B], F32)
    nc.vector.memset(onesC_f[:], float(C))
    onesC = consts.tile([1, B], BF16)
    nc.vector.tensor_copy(onesC[:], onesC_f[:])
    ones1_f = consts.tile([1, B], F32)
    nc.vector.memset(ones1_f[:], 1.0)
    ones1 = consts.tile([1, B], BF16)
    nc.vector.tensor_copy(ones1[:], ones1_f[:])
    bv_sb = consts.tile([1, D], BF16)
    nc.vector.tensor_copy(bv_sb[:], bv_f[:])
    bo_sb = consts.tile([1, EK], BF16)
    nc.vector.tensor_copy(bo_sb[:], bo_f[:])

    # --- phase A: fold C (bf16, DVE/GPSIMD alternating) + transposes ------
    a01s = [sbuf.tile([P, CD], BF16, name=f"a01_{i}") for i in range(2)]
    a23s = [sbuf.tile([P, CD], BF16, name=f"a23_{i}") for i in range(2)]
    acc = sbuf.tile([P, D], BF16)
    cst = sbuf.tile([P, 16 * B], BF16)  # csT block j at cols [64j, 64j+64)
    for c in range(NCC):
        eng = nc.vector if c % 2 == 0 else nc.gpsimd
        a01, a23 = a01s[c % 2], a23s[c % 2]
        base = c * 4 * CD
        eng.tensor_add(
            a01[:], ctrl_sb[:, base : base + CD], ctrl_sb[:, base + CD : base + 2 * CD]
        )
        eng.tensor_add(
            a23[:],
            ctrl_sb[:, base + 2 * CD : base + 3 * CD],
            ctrl_sb[:, base + 3 * CD : base + 4 * CD],
        )
        eng.tensor_add(acc[:, c * CD : (c + 1) * CD], a01[:], a23[:])
        if c == 0:
            # PE warm-up: dummy bf16 matmuls on already-resident data pull
            # the HAM clock gate to 2.4 GHz before the real GEMMs.
            pd = psum_d.tile([B, PW], F32, tag="pd")
            for i in range(9):
                nc.tensor.matmul(
                    pd[:],
                    ident_t[:],
                    ctrl_sb[0:B, 0:PW],
                    start=(i == 0),
                    stop=(i == 8),
                )
        for h in range(CD // P):
            j = c * (CD // P) + h
            pt = psum_t.tile([P, B], F32, tag="pt")
            nc.tensor.matmul(
                pt[:], acc[:, j * P : (j + 1) * P], sel[:], start=True, stop=True
            )
            if j % 2 == 0:
                nc.vector.tensor_copy(cst[:, j * B : (j + 1) * B], pt[:])
            else:
                nc.scalar.activation(cst[:, j * B : (j + 1) * B], pt[:], COPY)

    # --- phase B: per Wv column panel: MM1, bias, vT, MM2 ----------------
    pp = psum_p.tile([P, P], F32, tag="pp")  # p; partition = 64*eh + b
    for c in range(NPANEL):
        pv = psum_v.tile([B, PW], F32, tag="pv", name=f"pv{c}")
        w = wv_sb[c]
        for j in range(16):
            nc.tensor.matmul(
                pv[:],
                cst[:, j * B : (j + 1) * B],
                w[:, j * PW : (j + 1) * PW],
                start=(j == 0),
                stop=False,
            )
        nc.tensor.matmul(  # += C * bv (panel slice)
            pv[:], onesC[:], bv_sb[:, c * PW : (c + 1) * PW], start=False, stop=True
        )
        v = sbuf.tile([B, PW], BF16, name=f"v{c % 2}")
        nc.scalar.activation(v[:], pv[:], COPY)  # ACT: PSUM f32 -> bf16
        for h in range(4):
            t = 4 * c + h
            pt = psum_t.tile([P, B], BF16, name="ptv", tag="pt")
            nc.tensor.transpose(pt[:], v[:, h * P : (h + 1) * P], ident_t[:])
            vt = sbuf.tile([P, B], BF16, name=f"vt{t % 4}")
            nc.scalar.activation(vt[:], pt[:], COPY)
            for half in range(2):
                nc.tensor.matmul(
                    pp[half * B : (half + 1) * B, :],
                    vt[:],
                    wo_sb[:, t * EK + half * P : t * EK + (half + 1) * P],
                    start=(t == 0),
                    stop=False,
                )
    for half in range(2):  # += 1 * bo
        nc.tensor.matmul(
            pp[half * B : (half + 1) * B, :],
            ones1[:],
            bo_sb[:, half * P : (half + 1) * P],
            start=False,
            stop=(half == 1),
        )

    # --- tail: out = seq + broadcast_s(p), 8 chunks, DVE/GPSIMD split -----
    p_re = sbuf.tile([P, P], F32)
    nc.scalar.activation(p_re[:], pp[:], COPY)
    NOUT = 8
    W = S * P // NOUT  # 512 (4 s-steps)
    out_sb = sbuf.tile([P, S * P], F32)
    for c in range(NOUT):
        sl = slice(c * W, (c + 1) * W)
        nc.scalar.activation(out_sb[:, sl], seq_sb[:, sl], COPY)  # fp8 -> f32
    for c in range(NOUT):
        sl = slice(c * W, (c + 1) * W)
        eng = nc.vector if c % 2 == 0 else nc.gpsimd
        eng.tensor_add(
            out_sb[:, sl].rearrange("p (s e) -> p s e", e=P),
            out_sb[:, sl].rearrange("p (s e) -> p s e", e=P),
            p_re[:, None, :].to_broadcast((P, S // NOUT, P)),
        )
        nc.scalar.dma_start(out=out[:, sl], in_=out_sb[:, sl])
    ctx.close()


def _get_nc():
    if "nc" not in _CACHE:
        _CACHE["nc"] = _build_nc()
    return _CACHE["nc"]


def _pack_rows(a):
    """[T*128, F] -> [128, T*F]: partition-major SBUF layout, contiguous."""
    T = a.shape[0] // P
    return np.ascontiguousarray(
        a.reshape(T, P, a.shape[1]).transpose(1, 0, 2).reshape(P, T * a.shape[1])
    )


def _shard(sequence, controls, Wv, bv, Wo, bo):
    bf = ml_dtypes.bfloat16
    f8 = ml_dtypes.float8_e4m3
    # ctrl: [512, 2048] -> chunks of 512 d-cols, each [128, 4 x 512]
    cb = controls.reshape(C * B, D).astype(bf)
    ctrl = np.ascontiguousarray(
        cb.reshape(4, P, NCC, CD).transpose(1, 2, 0, 3).reshape(P, 4 * D)
    )
    # Wv.T column panels, each packed to [128, 16*PW]
    wvt = Wv.T.astype(bf)  # [d, f]
    wvp = np.ascontiguousarray(
        np.concatenate(
            [_pack_rows(wvt[:, cc * PW : (cc + 1) * PW]) for cc in range(NPANEL)],
            axis=0,
        )
    )
    bvr = np.ascontiguousarray(bv[None, :].astype(np.float32))
    in_maps = []
    for k in range(N_CORES):
        sl = slice(k * EK, (k + 1) * EK)
        in_maps.append(
            {
                "seq": np.ascontiguousarray(
                    sequence[:, :, sl]
                    .reshape(B, S, 2, P)
                    .transpose(2, 0, 1, 3)
                    .reshape(P, S * P)
                    .astype(f8)
                ),
                "ctrl": ctrl,
                "wvp": wvp,
                "wot": _pack_rows(Wo[sl, :].T.astype(bf)),
                "bv": bvr,
                "bo": np.ascontiguousarray(bo[None, sl].astype(np.float32)),
            }
        )
    return in_maps


def _run(inputs, trace=False):
    nc = _get_nc()
    in_maps = _shard(
        np.asarray(inputs["sequence"]), np.asarray(inputs["controls"]),
        np.asarray(inputs["Wv"]), np.asarray(inputs["bv"]),
        np.asarray(inputs["Wo"]), np.asarray(inputs["bo"]),
    )
    res = run_bass_kernel_spmd(nc, in_maps, list(range(N_CORES)), trace=trace)
    out = np.empty((B, S, D), dtype=np.float32)
    for k in range(N_CORES):
        out[:, :, k * EK : (k + 1) * EK] = (
            res.results[k]["out"]
            .reshape(2, B, S, P)
            .transpose(1, 2, 0, 3)
            .reshape(B, S, EK)
        )
    return out, res


def kernel(**inputs):
    out, _ = _run(inputs)
    return out


# revision 12
# speedup vs baseline: 1.0136x; 1.0136x over previous
"""Trainium2 Bass kernel for nn_CompactControlAttention.

The module's attention is degenerate: softmax over a size-1 axis is exactly
1.0, so queries/keys (Wq, bq, Wk, bk) never affect the output:

    out[b, s, :] = sequence[b, s, :] + p[b, :]
    p = (sum_c controls[c]) @ Wv.T @ Wo.T + C * (bv @ Wo.T + bo)

Sharding (8 cores, no collectives): tensor-parallel over the OUTPUT feature
dim e.  Core k computes out[:, :, 256k:256(k+1)], needing full Wv
(replicated), a 256-column slice of Wo, and matching slices of seq/bo.
Cross-core collectives were measured to pay a 50-70us first-collective
rendezvous on this runtime (core launch skew), so the kernel is
communication-free and optimized as a single streaming pipeline:

  - All tensors are pre-packed host-side into SBUF layout ([128, free],
    partition-contiguous) so every DMA moves at line rate (~410 GB/s).
  - ctrl (bf16) streams in 4 sub-chunks; the fold over C (bf16 pair-adds,
    alternating DVE/GPSIMD) and the stacked-identity transpose-matmul
    (folds the last c-parity pair) pipeline against the stream.
  - A dummy-matmul burst right after the first ctrl chunk pulls the PE out
    of the HAM 1.2 GHz throttle before the real GEMM work arrives.
  - Wv.T streams as 4 column panels x 4 sub-chunks; MM1 K-tiles fire per
    sub-chunk.  Each panel's bias/cast/transposes/MM2 overlap the next
    panel's stream.  PSUM is quad-buffered so no panel ever waits on a
    downstream consumer.
  - Engine split: PE does all matmuls/transposes; DVE and GPSIMD split the
    folds and the final residual adds; ACT does every PSUM->SBUF cast (v,
    csT blocks, p) plus the seq fp8->f32 casts, interleaved into the panel
    loop; output writes stream on the scalar (ACT HWDGE) queue.

Precision: bf16 weights/activations with fp32 PSUM accumulation; sequence
ships as fp8e4m3 (it only enters the final residual add; adds ~1e-3 of
output-scale rounding).  Measured rel err ~4e-3 vs the fp32 reference.
"""

import numpy as np
import ml_dtypes

import concourse.bass as bass
import concourse.mybir as mybir
import concourse.tile as tile
from concourse import bacc
from concourse.bass_utils import run_bass_kernel_spmd
from concourse.masks import make_identity

N_CORES = 8
D = 2048
B = 64
S = 32
C = 8
EK = D // N_CORES  # 256: per-core output-feature slice
F32 = mybir.dt.float32
BF16 = mybir.dt.bfloat16
FP8 = mybir.dt.float8e4
P = 128
NPANEL = 4
PW = D // NPANEL  # 512 v-features per panel
NSUB = 4  # sub-DMAs per panel (4 K-tiles each)
NCC = 4  # ctrl sub-chunks (512 d-cols each)
CD = D // NCC  # 512
COPY = mybir.ActivationFunctionType.Copy

_CACHE = {}


def _build_nc():
    nc = bacc.Bacc("TRN2", target_bir_lowering=False, debug=False, num_devices=N_CORES)

    # All inputs pre-packed host-side to SBUF layout [128, free].
    seq = nc.dram_tensor("seq", [P, S * P], FP8, kind="ExternalInput")
    ctrl = nc.dram_tensor("ctrl", [P, 4 * D], BF16, kind="ExternalInput")
    wvp = nc.dram_tensor("wvp", [NPANEL * P, 16 * PW], BF16, kind="ExternalInput")
    wot = nc.dram_tensor("wot", [P, 16 * EK], BF16, kind="ExternalInput")
    bv = nc.dram_tensor("bv", [1, D], F32, kind="ExternalInput")
    bo = nc.dram_tensor("bo", [1, EK], F32, kind="ExternalInput")
    out = nc.dram_tensor("out", [P, S * P], F32, kind="ExternalOutput")

    with tile.TileContext(nc) as tc:
        _body(tc, seq, ctrl, wvp, wot, bv, bo, out)
    nc.compile()
    return nc


def _body(tc, seq, ctrl, wvp, wot, bv, bo, out):
    from contextlib import ExitStack

    ctx = ExitStack()
    nc = tc.nc

    consts = ctx.enter_context(tc.tile_pool(name="consts", bufs=1))
    sbuf = ctx.enter_context(tc.tile_pool(name="sbuf", bufs=1))
    wpool = ctx.enter_context(tc.tile_pool(name="wv", bufs=2))
    psum_t = ctx.enter_context(tc.tile_pool(name="psum_t", bufs=2, space="PSUM"))
    psum_v = ctx.enter_context(tc.tile_pool(name="psum_v", bufs=4, space="PSUM"))
    psum_p = ctx.enter_context(tc.tile_pool(name="psum_p", bufs=1, space="PSUM"))
    psum_d = ctx.enter_context(tc.tile_pool(name="psum_d", bufs=1, space="PSUM"))

    # --- constants: tiny DMAs at the FRONT of the sync queue --------------
    ident = consts.tile([P, P], F32)
    make_identity(nc, ident[:])
    sel_f = consts.tile([P, B], F32)  # two stacked 64x64 identities
    nc.sync.dma_start(out=sel_f[0:B, :], in_=ident[0:B, 0:B])
    nc.sync.dma_start(out=sel_f[B : 2 * B, :], in_=ident[0:B, 0:B])
    bv_f = consts.tile([1, D], F32)
    nc.sync.dma_start(out=bv_f[:], in_=bv[:])
    bo_f = consts.tile([1, EK], F32)
    nc.sync.dma_start(out=bo_f[:], in_=bo[:])

    # --- sync (SP) queue: ctrl x4, p0s0, p0s1, wot, p0s2.., seq on gpsimd -
    # Separate tiles per chunk/sub-DMA: dependency tracking is
    # tile-granular, so consumers must not share a tile with later DMAs.
    ctrl_sb = [sbuf.tile([P, 4 * CD], BF16, name=f"ctrl{c}") for c in range(NCC)]
    for c in range(NCC):
        nc.sync.dma_start(
            out=ctrl_sb[c][:], in_=ctrl[:, c * 4 * CD : (c + 1) * 4 * CD]
        )

    wsub = [[None] * NSUB for _ in range(NPANEL)]
    wo_sb = sbuf.tile([P, 16 * EK], BF16)
    for c in range(NPANEL):
        for s in range(NSUB):
            w = sbuf.tile([P, 4 * PW], BF16, name=f"ws{(c % 2) * NSUB + s}")
            wsub[c][s] = w
            nc.sync.dma_start(
                out=w[:],
                in_=wvp[c * P : (c + 1) * P, s * 4 * PW : (s + 1) * 4 * PW],
            )
            if c == 0 and s == 1:  # wot mid-panel-0: arrives before MM2-p0
                nc.sync.dma_start(out=wo_sb[:], in_=wot[:])
    seq_sb = sbuf.tile([P, S * P], FP8)
    nc.gpsimd.dma_start(out=seq_sb[:], in_=seq[:])

    # --- small on-engine constants ---------------------------------------
    sel = consts.tile([P, B], BF16)
    nc.vector.tensor_copy(sel[:], sel_f[:])
    ident_t = consts.tile([B, B], BF16)
    nc.vector.tensor_copy(ident_t[:], ident[0:B, 0:B])
    onesC_f = consts.tile([1, B], F32)
    nc.vector.memset(onesC_f[:], float(C))
    onesC = consts.tile([1, B], BF16)
    nc.vector.tensor_copy(onesC[:], onesC_f[:])
    ones1_f = consts.tile([1, B], F32)
    nc.vector.memset(ones1_f[:], 1.0)
    ones1 = consts.tile([1, B], BF16)
    nc.vector.tensor_copy(ones1[:], ones1_f[:])
    bv_sb = consts.tile([1, D], BF16)
    nc.vector.tensor_copy(bv_sb[:], bv_f[:])
    bo_sb = consts.tile([1, EK], BF16)
    nc.vector.tensor_copy(bo_sb[:], bo_f[:])

    # --- phase A: fold C (bf16, DVE/GPSIMD alternating) + transposes ------
    a01s = [sbuf.tile([P, CD], BF16, name=f"a01_{i}") for i in range(2)]
    a23s = [sbuf.tile([P, CD], BF16, name=f"a23_{i}") for i in range(2)]
    acc = sbuf.tile([P, D], BF16)
    cst = sbuf.tile([P, 16 * B], BF16)  # csT block j at cols [64j, 64j+64)
    for c in range(NCC):
        cb = ctrl_sb[c]
        a01, a23 = a01s[c % 2], a23s[c % 2]
        nc.vector.tensor_add(a01[:], cb[:, 0:CD], cb[:, CD : 2 * CD])
        nc.gpsimd.tensor_add(a23[:], cb[:, 2 * CD : 3 * CD], cb[:, 3 * CD : 4 * CD])
        nc.vector.tensor_add(acc[:, c * CD : (c + 1) * CD], a01[:], a23[:])
        if c == 0:
            # PE warm-up: dummy bf16 matmuls on already-resident data pull
            # the HAM clock gate to 2.4 GHz before the real GEMMs.
            pd = psum_d.tile([B, PW], F32, tag="pd")
            for i in range(9):
                nc.tensor.matmul(
                    pd[:],
                    ident_t[:],
                    cb[0:B, 0:PW],
                    start=(i == 0),
                    stop=(i == 8),
                )
        for h in range(CD // P):
            j = c * (CD // P) + h
            pt = psum_t.tile([P, B], F32, tag="pt")
            nc.tensor.matmul(
                pt[:], acc[:, j * P : (j + 1) * P], sel[:], start=True, stop=True
            )
            if j % 2 == 0:
                nc.vector.tensor_copy(cst[:, j * B : (j + 1) * B], pt[:])
            else:
                nc.scalar.activation(cst[:, j * B : (j + 1) * B], pt[:], COPY)

    # --- phase B: per Wv column panel: MM1, bias, vT, MM2 ----------------
    pp = psum_p.tile([P, P], F32, tag="pp")  # p; partition = 64*eh + b
    for c in range(NPANEL):
        pv = psum_v.tile([B, PW], F32, tag="pv", name=f"pv{c}")
        for j in range(16):
            nc.tensor.matmul(
                pv[:],
                cst[:, j * B : (j + 1) * B],
                wsub[c][j // 4][:, (j % 4) * PW : (j % 4 + 1) * PW],
                start=(j == 0),
                stop=False,
            )
        nc.tensor.matmul(  # += C * bv (panel slice)
            pv[:], onesC[:], bv_sb[:, c * PW : (c + 1) * PW], start=False, stop=True
        )
        v = sbuf.tile([B, PW], BF16, name=f"v{c % 2}")
        nc.scalar.activation(v[:], pv[:], COPY)  # ACT: PSUM f32 -> bf16
        for h in range(4):
            t = 4 * c + h
            pt = psum_t.tile([P, B], BF16, name="ptv", tag="pt")
            nc.tensor.transpose(pt[:], v[:, h * P : (h + 1) * P], ident_t[:])
            vt = sbuf.tile([P, B], BF16, name=f"vt{t % 4}")
            nc.scalar.activation(vt[:], pt[:], COPY)
            for half in range(2):
                nc.tensor.matmul(
                    pp[half * B : (half + 1) * B, :],
                    vt[:],
                    wo_sb[:, t * EK + half * P : t * EK + (half + 1) * P],
                    start=(t == 0),
                    stop=False,
                )
    for half in range(2):  # += 1 * bo
        nc.tensor.matmul(
            pp[half * B : (half + 1) * B, :],
            ones1[:],
            bo_sb[:, half * P : (half + 1) * P],
            start=False,
            stop=(half == 1),
        )

    # --- tail: out = seq + broadcast_s(p), 8 chunks, DVE/GPSIMD split -----
    p_re = sbuf.tile([P, P], F32)
    nc.scalar.activation(p_re[:], pp[:], COPY)
    NOUT = 8
    W = S * P // NOUT  # 512 (4 s-steps)
    # expand p once so the per-chunk adds are plain 2D ops
    p_wide = sbuf.tile([P, W], F32)
    nc.vector.tensor_copy(
        p_wide[:].rearrange("p (s e) -> p s e", e=P),
        p_re[:, None, :].to_broadcast((P, W // P, P)),
    )
    out_sb = sbuf.tile([P, S * P], F32)
    for c in range(NOUT):
        sl = slice(c * W, (c + 1) * W)
        nc.scalar.activation(out_sb[:, sl], seq_sb[:, sl], COPY)  # fp8 -> f32
    for c in range(NOUT):
        sl = slice(c * W, (c + 1) * W)
        eng = nc.gpsimd if c in (5, 6, 7) else nc.vector
        eng.tensor_add(out_sb[:, sl], out_sb[:, sl], p_wide[:])
        nc.scalar.dma_start(out=out[:, sl], in_=out_sb[:, sl])
    ctx.close()


def _get_nc():
    if "nc" not in _CACHE:
        _CACHE["nc"] = _build_nc()
    return _CACHE["nc"]


def _pack_rows(a):
    """[T*128, F] -> [128, T*F]: partition-major SBUF layout, contiguous."""
    T = a.shape[0] // P
    return np.ascontiguousarray(
        a.reshape(T, P, a.shape[1]).transpose(1, 0, 2).reshape(P, T * a.shape[1])
    )


def _shard(sequence, controls, Wv, bv, Wo, bo):
    bf = ml_dtypes.bfloat16
    f8 = ml_dtypes.float8_e4m3
    # ctrl: [512, 2048] -> chunks of 512 d-cols, each [128, 4 x 512]
    cb = controls.reshape(C * B, D).astype(bf)
    ctrl = np.ascontiguousarray(
        cb.reshape(4, P, NCC, CD).transpose(1, 2, 0, 3).reshape(P, 4 * D)
    )
    # Wv.T column panels, each packed to [128, 16*PW]
    wvt = Wv.T.astype(bf)  # [d, f]
    wvp = np.ascontiguousarray(
        np.concatenate(
            [_pack_rows(wvt[:, cc * PW : (cc + 1) * PW]) for cc in range(NPANEL)],
            axis=0,
        )
    )
    bvr = np.ascontiguousarray(bv[None, :].astype(np.float32))
    in_maps = []
    for k in range(N_CORES):
        sl = slice(k * EK, (k + 1) * EK)
        in_maps.append(
            {
                "seq": np.ascontiguousarray(
                    sequence[:, :, sl]
                    .reshape(B, S, 2, P)
                    .transpose(2, 0, 1, 3)
                    .reshape(P, S * P)
                    .astype(f8)
                ),
                "ctrl": ctrl,
                "wvp": wvp,
                "wot": _pack_rows(Wo[sl, :].T.astype(bf)),
                "bv": bvr,
                "bo": np.ascontiguousarray(bo[None, sl].astype(np.float32)),
            }
        )
    return in_maps


def _run(inputs, trace=False):
    nc = _get_nc()
    in_maps = _shard(
        np.asarray(inputs["sequence"]), np.asarray(inputs["controls"]),
        np.asarray(inputs["Wv"]), np.asarray(inputs["bv"]),
        np.asarray(inputs["Wo"]), np.asarray(inputs["bo"]),
    )
    res = run_bass_kernel_spmd(nc, in_maps, list(range(N_CORES)), trace=trace)
    out = np.empty((B, S, D), dtype=np.float32)
    for k in range(N_CORES):
        out[:, :, k * EK : (k + 1) * EK] = (
            res.results[k]["out"]
            .reshape(2, B, S, P)
            .transpose(1, 2, 0, 3)
            .reshape(B, S, EK)
        )
    return out, res


def kernel(**inputs):
    out, _ = _run(inputs)
    return out
